# revision 21
# baseline (speedup 1.0000x reference)
"""Trainium2 Bass kernel for nn_EncoderBlock (pre-norm self-attention + FFN).

Sharding (8 cores): core c -> batch b = c//4, head-group j = c%4
(heads 4j..4j+3, Wq columns 256j..256j+256, Wo rows 256j..256j+256),
token slice 256j..256j+256 for the post-attention residual + FFN part.
One ReduceScatter (the "all-reduce after Wo" of the TP hint, fused with
the token scatter) inside each 4-core batch group; everything else is
local.

Key tricks:
 - LayerNorm1 folded into the QKV matmuls via an augmented contraction row
   (lhsT rows: [Wq*ln1_a ; colsum(Wq*ln1_a)], rhs rows: [x^T ; -mu]) and a
   1/(std+eps) column/row scale fused into the PSUM evacuation.
 - Scores are computed directly in [k, q] layout (q==k==v here, so both
   operands come from the same qkvT tile).  masked_fill(mask==0, 1e-9) ~=
   multiply scores by the 0/1 mask (exp-side error ~1e-9).
 - Softmax without max-subtraction (scores are O(+-5) so exp is safe) and
   without a separate denominator pass: a ones column is appended to the
   attn@v lhsT, so PSUM row 64 of the context matmul is the softmax Z.
 - Normalization by 1/Z (a per-q, i.e. per-free-column, factor) uses a
   K=4 selector matmul on the PE to broadcast rows across partitions.
 - FFN is token-sharded with full weights; LN2 scale/bias folded into W1
   (ln2_a) and the relu bias (W1^T ln2_b + B1); B2 applied via an extra
   ones contraction row on the second FFN matmul.

All big matmuls run in float32r (TF32-like, full PE rate at N>=256).
"""

import numpy as np
import ml_dtypes

import concourse.bass as bass
import concourse.mybir as mybir
import concourse.tile as tile
from concourse import bacc
from concourse import bass_utils
from concourse.masks import make_identity

F32 = mybir.dt.float32
F32R = mybir.dt.float32r
BF16 = mybir.dt.bfloat16
AF = mybir.ActivationFunctionType
MULT = mybir.AluOpType.mult
ADD = mybir.AluOpType.add
SUB = mybir.AluOpType.subtract

B, S, D, H, DK, DFF = 2, 1024, 1024, 16, 64, 4096
EPS = 1e-6
P = 128
NC = 8
KS = D // P            # 8 k-subtiles over d_model
KA = KS + 1            # + augmented subtile (row 0 = -mu)
FFS = DFF // P         # 32 ff subtiles
FFA = FFS + 1          # + augmented subtile (row 0 = ones -> B2)
TS = S // 4            # 256-token slice per core
TM = TS // P           # 2 token M-tiles
SM = S // P            # 8 token M-tiles (full sequence)
HD = 4                 # heads per core
HCOLS = HD * DK        # 256 qkv columns per core
HM = HCOLS // P        # 2 partition strips of qkv columns
GROUPS = [[0, 1, 2, 3], [4, 5, 6, 7]]

_CACHE = {}


def _build():
    nc = bacc.Bacc("TRN2", target_bir_lowering=False, debug=False, num_devices=NC)

    def din(name, shape, dt):
        return nc.dram_tensor(name, shape, dt, kind="ExternalInput")

    xt = din("xt", [P, KA, S], BF16)          # x[b]^T striped + aug subtile
    wq = din("wq", [P, KA, HCOLS], BF16)      # (Wq*a1) cols + g row, striped
    ones1 = din("ones1", [1, P], F32R)        # ones row for partition-bcast
    sel4 = din("sel4", [HD, HCOLS], F32R)     # head-selector for Z broadcast
    c1r = din("c1r", [1, HCOLS], F32R)        # Wq_cols^T @ ln1_b (row form)
    c1c = din("c1c", [P, HM], F32)            # same, column form [128, 2]
    xsl = din("xsl", [P, TM, D], F32)         # x token-slice (natural)
    maskt = din("maskt", [P, KS, S], BF16)    # mask[b,0]^T striped
    wo = din("wo", [P, HM, D], BF16)          # Wo rows 256j..256j+256, striped
    w1 = din("w1", [P, KS, DFF], BF16)        # W1*a2 striped (lhsT)
    w2 = din("w2", [P, FFA, D], BF16)         # [W2 ; B2 ; 0-pad] striped (rhs)
    bias1 = din("bias1", [P, FFS], F32)       # B1 + W1^T ln2_b, striped cols
    fftail = din("fftail", [P, TS], BF16)     # relu aug tail: row0=ones
    out = nc.dram_tensor("out", [TS, D], F32, kind="ExternalOutput")

    with tile.TileContext(nc) as tc:
        with (
            tc.tile_pool(name="glob", bufs=1) as glob,
            tc.tile_pool(name="gdram", bufs=1, space="DRAM") as gdram,
        ):
            # ---- tiles that cross phases ----
            qkvT = glob.tile([P, HM, S], BF16)           # [d'(2 strips), tok]
            qkv = glob.tile([P, KS, 68 * HD], BF16)      # per-head 64+ones+3pad
            ctxTu = glob.tile([P, HM, S], F32)           # unnormalized ctx^T
            zrow = glob.tile([1, HD * S], F32R)          # Z per head (packed)
            ctxn = glob.tile([P, HM, S], BF16)           # normalized ctx^T
            ones1t = glob.tile([1, P], F32R)
            nc.sync.dma_start(ones1t[:], ones1[:])
            sel4t = glob.tile([HD, HCOLS], F32R)
            nc.sync.dma_start(sel4t[:], sel4[:])
            c1ct = glob.tile([P, HM], F32)
            nc.sync.dma_start(c1ct[:], c1c[:])
            r1col = glob.tile([P, KS], F32)              # 1/(std+eps) per tok
            scr = gdram.tile([P, KS], F32)               # round-trip scratch
            scr2 = gdram.tile([HD, S], F32R)             # rz round-trip
            bounce_inA = gdram.tile([S // 2, D], BF16)   # attn-out partial, even
            bounce_inB = gdram.tile([S // 2, D], BF16)   # attn-out partial, odd
            bounce_rsA = gdram.tile([P, D], BF16)        # my tokens 0:128, summed
            bounce_rsB = gdram.tile([P, D], BF16)        # my tokens 128:256

            # ================= Phase A: LN1 stats + QKV =================
            with tc.tile_pool(name="pha", bufs=1) as pha:
                psA_cm = tc.tile_pool(name="psA", bufs=1, space="PSUM")
                psA = psA_cm.__enter__()
                xtt = pha.tile([P, KA, S], BF16)
                for k in range(KA):
                    nc.sync.dma_start(xtt[:, k], xt[:, k])
                wqt = pha.tile([P, KA, HCOLS], BF16)
                nc.sync.dma_start(wqt[:], wq[:])
                c1rt = pha.tile([1, HCOLS], F32R)
                nc.sync.dma_start(c1rt[:], c1r[:])

                xsq = pha.tile([P, KS, S], BF16)
                for k in range(KS):
                    nc.scalar.activation(xsq[:, k], xtt[:, k], AF.Square)
                ones16 = pha.tile([P, KS, 1], BF16)
                nc.gpsimd.memset(ones16[:], 1.0)
                zero16 = pha.tile([P, 1, 1], BF16)
                nc.gpsimd.memset(zero16[:], 0.0)

                ps_s1 = psA.tile([1, S], F32, name="ps_s1", tag="ps_a")
                ps_s2 = psA.tile([1, S], F32, name="ps_s2", tag="ps_b")
                for nb in range(2):
                    qs = slice(512 * nb, 512 * nb + 512)
                    for k in range(KS):
                        nc.tensor.matmul(
                            ps_s1[:, qs], ones16[:, k], xtt[:, k, qs],
                            start=(k == 0), stop=(k == KS - 1),
                        )
                    for k in range(KS):
                        nc.tensor.matmul(
                            ps_s2[:, qs], ones16[:, k], xsq[:, k, qs],
                            start=(k == 0), stop=(k == KS - 1),
                        )

                # -mu into the aug row of xt (read by the qkv matmuls below)
                nc.vector.tensor_scalar_mul(xtt[0:1, KS, :], ps_s1[:], -1.0 / D)

                # r1 = 1/(std+eps), std = sqrt((S2 - S1^2/D)/(D-1))
                s1s = pha.tile([1, S], F32)
                nc.vector.tensor_copy(s1s[:], ps_s1[:])
                tvar = pha.tile([1, S], F32)
                nc.vector.tensor_tensor(tvar[:], s1s[:], s1s[:], MULT)
                nc.vector.tensor_scalar_mul(tvar[:], tvar[:], -1.0 / D)
                nc.vector.tensor_tensor(tvar[:], tvar[:], ps_s2[:], ADD)
                nc.vector.tensor_scalar_mul(tvar[:], tvar[:], 1.0 / (D - 1))
                stdr = pha.tile([1, S], F32)
                nc.scalar.activation(stdr[:], tvar[:], AF.Sqrt)
                nc.vector.tensor_scalar_add(stdr[:], stdr[:], EPS)
                r1r = pha.tile([1, S], F32R)
                nc.vector.tensor_copy(r1r[:], stdr[:])

                # column layout of std+eps via DRAM round-trip, then a
                # 128-lane reciprocal (1-lane recips are ~5.5us each)
                nc.sync.dma_start(
                    scr[:].rearrange("p o -> o p").unsqueeze(0),
                    stdr[0:1, :].rearrange("one (o p) -> one o p", o=KS),
                )
                stdcol = pha.tile([P, KS], F32)
                nc.sync.dma_start(stdcol[:], scr[:])
                nc.vector.reciprocal(r1col[:], stdcol[:])

                # broadcast r1 / c1 across partitions via K=1 matmuls
                ps_r1 = psA.tile([P, S], F32, name="ps_r1", tag="ps_a")
                for nb in range(2):
                    qs = slice(512 * nb, 512 * nb + 512)
                    nc.tensor.matmul(
                        ps_r1[:, qs], ones1t[:], r1r[:, qs],
                        start=True, stop=True,
                    )
                R1 = pha.tile([P, S], F32, tag="bcastbuf", padded_shape=None)
                nc.vector.reciprocal(R1[:], ps_r1[:])
                ps_c1 = psA.tile([P, HCOLS], F32, name="ps_c1", tag="ps_b")
                nc.tensor.matmul(
                    ps_c1[:], ones1t[:], c1rt[:], start=True, stop=True
                )
                C1b = pha.tile([P, HCOLS], F32)
                nc.vector.tensor_copy(C1b[:], ps_c1[:])

                # qkvT[d', tok] = (Wa^T x - g mu) * r1 + c1
                for m in range(HM):
                    ps_qt = psA.tile([P, S], F32, name="ps_qt", tag="ps_mm",
                                     bufs=2)
                    for nb in range(2):
                        qs = slice(512 * nb, 512 * nb + 512)
                        for k in range(KA):
                            nc.tensor.matmul(
                                ps_qt[:, qs],
                                wqt[:, k, m * P:(m + 1) * P],
                                xtt[:, k, qs],
                                start=(k == 0), stop=(k == KA - 1),
                            )
                    nc.vector.tensor_tensor(qkvT[:, m, :], ps_qt[:], R1[:], MULT)
                    nc.vector.tensor_tensor(
                        qkvT[:, m, :], qkvT[:, m, :],
                        c1ct[:, m:m + 1].to_broadcast((P, S)), ADD,
                    )

                # qkv[tok, d'] natural layout (+ ones cols for the Z row)
                for h in range(HD):
                    nc.gpsimd.memset(qkv[:, :, 68 * h + 64:68 * h + 65], 1.0)
                for m in range(KS):
                    ps_q = psA.tile([P, HCOLS], F32, name="ps_q", tag="ps_mm",
                                    bufs=2)
                    for k in range(KA):
                        nc.tensor.matmul(
                            ps_q[:], xtt[:, k, m * P:(m + 1) * P], wqt[:, k],
                            start=(k == 0), stop=(k == KA - 1),
                        )
                    for h in range(HD):
                        nc.vector.tensor_tensor(
                            qkv[:, m, 68 * h:68 * h + 64],
                            ps_q[:, 64 * h:64 * h + 64],
                            r1col[:, m:m + 1].to_broadcast((P, DK)),
                            MULT,
                        )
                        nc.vector.tensor_tensor(
                            qkv[:, m, 68 * h:68 * h + 64],
                            qkv[:, m, 68 * h:68 * h + 64],
                            C1b[:, 64 * h:64 * h + 64],
                            ADD,
                        )

                psA_cm.__exit__(None, None, None)

                # ============= Phase B: attention per head =============
                with (
                    tc.tile_pool(name="phb", bufs=1) as phb,
                    tc.tile_pool(name="psB", bufs=1, space="PSUM") as psB,
                ):
                    masktt = phb.tile([P, KS, S], BF16)
                    nc.sync.dma_start(masktt[:], maskt[:])
                    wot = pha.tile([P, HM, D], BF16)
                    nc.sync.dma_start(wot[:], wo[:])
                    for pair in range(2):
                        eTs = {}
                        for h in (2 * pair, 2 * pair + 1):
                            hp = 64 * (h % 2)
                            hs = h // 2
                            eT = pha.tile([P, KS, S], BF16, tag="bigbuf16",
                                          name=f"eT{h}", bufs=2)
                            eTs[h] = eT
                            for m in range(KS):
                                for nb in range(2):
                                    qs = slice(512 * nb, 512 * nb + 512)
                                    ps_sc = psB.tile(
                                        [P, 512], F32, name="ps_sc", bufs=4
                                    )
                                    nc.tensor.matmul(
                                        ps_sc[:],
                                        qkvT[hp:hp + 64, hs,
                                             m * P:(m + 1) * P],
                                        qkvT[hp:hp + 64, hs, qs],
                                        start=True, stop=True,
                                    )
                                    etmp = phb.tile(
                                        [P, 512], F32, name="etmp", bufs=4
                                    )
                                    nc.vector.tensor_tensor(
                                        etmp[:], ps_sc[:], masktt[:, m, qs],
                                        MULT,
                                    )
                                    nc.scalar.activation(
                                        eT[:, m, qs], etmp[:], AF.Exp,
                                        scale=float(1.0 / np.sqrt(DK)),
                                    )
                        for h in (2 * pair, 2 * pair + 1):
                            hp = 64 * (h % 2)
                            hs = h // 2
                            eT = eTs[h]
                            ps_ct = psB.tile([P, S], F32, name="ps_ct",
                                             bufs=2)
                            for nb in range(2):
                                qs = slice(512 * nb, 512 * nb + 512)
                                for k in range(KS):
                                    nc.tensor.matmul(
                                        ps_ct[0:65, qs],
                                        qkv[:, k, 68 * h:68 * h + 65],
                                        eT[:, k, qs],
                                        start=(k == 0), stop=(k == KS - 1),
                                    )
                            nc.vector.tensor_copy(
                                zrow[0:1, S * h:S * h + S], ps_ct[64:65, :]
                            )
                            nc.sync.dma_start(
                                scr2[h:h + 1, :],
                                zrow[0:1, S * h:S * h + S],
                            )
                            nc.vector.tensor_copy(
                                ctxTu[hp:hp + 64, hs, :], ps_ct[0:64, :]
                            )

                # ====== Phase C: normalize + attn-out partial + RS ======
                with (
                    tc.tile_pool(name="phc", bufs=1) as phc,
                    tc.tile_pool(name="psC", bufs=1, space="PSUM") as psC,
                ):
                    zr4 = pha.tile([HD, S], F32R)
                    nc.sync.dma_start(zr4[:], scr2[:])
                    nrmb = pha.tile([P, HM, S], F32, tag="bcastbuf")
                    for m in range(HM):
                        ps_nb = psC.tile([P, S], F32, name="ps_nb", bufs=2)
                        for nb in range(2):
                            qs = slice(512 * nb, 512 * nb + 512)
                            nc.tensor.matmul(
                                ps_nb[:, qs],
                                sel4t[:, m * P:(m + 1) * P],
                                zr4[:, qs],
                                start=True, stop=True,
                            )
                        nc.vector.reciprocal(nrmb[:, m, :], ps_nb[:])
                        nc.vector.tensor_tensor(
                            ctxn[:, m, :], nrmb[:, m, :], ctxTu[:, m, :], MULT
                        )

                    # attn-out partial for ALL tokens from my heads:
                    # ao[t, d] = sum_{d' in mine} ctxn[d', t] * Wo[d', d]
                    aosb = pha.tile([P, SM, D], BF16, tag="bigbuf")
                    for mt in [0, 2, 4, 6, 1, 3, 5, 7]:
                        ps_ao = psC.tile([P, D], F32, name="ps_ao", bufs=2)
                        for nb in range(2):
                            ds_ = slice(512 * nb, 512 * nb + 512)
                            for k in range(HM):
                                nc.tensor.matmul(
                                    ps_ao[:, ds_],
                                    ctxn[:, k, mt * P:(mt + 1) * P],
                                    wot[:, k, ds_],
                                    start=(k == 0), stop=(k == HM - 1),
                                )
                        nc.vector.tensor_copy(aosb[:, mt, :], ps_ao[:])
                        if mt == 6:
                            nc.sync.dma_start(
                                bounce_inA[:].rearrange(
                                    "(r p) d -> p r d", p=P
                                ),
                                aosb[:, 0:SM:2, :],
                            )
                    nc.sync.dma_start(
                        bounce_inB[:].rearrange("(r p) d -> p r d", p=P),
                        aosb[:, 1:SM:2, :],
                    )

            nc.gpsimd.collective_compute(
                "ReduceScatter",
                mybir.AluOpType.add,
                replica_groups=GROUPS,
                ins=[bounce_inA.opt()],
                outs=[bounce_rsA.opt()],
            )
            nc.gpsimd.collective_compute(
                "ReduceScatter",
                mybir.AluOpType.add,
                replica_groups=GROUPS,
                ins=[bounce_inB.opt()],
                outs=[bounce_rsB.opt()],
            )

            # ========== Phase D: residual + LN2 + FFN ==========
            with (
                tc.tile_pool(name="phd", bufs=1) as phd,
                tc.tile_pool(name="psD", bufs=1, space="PSUM") as psD,
            ):
                aorss = []
                x2s = []
                n2s = []
                for m in range(TM):
                    aors = phd.tile([P, D], BF16, name=f"aors{m}")
                    nc.sync.dma_start(
                        aors[:], (bounce_rsA if m == 0 else bounce_rsB)[:]
                    )
                    xslt = phd.tile([P, D], F32, name=f"xslt{m}")
                    nc.sync.dma_start(xslt[:], xsl[:, m])
                    x2 = phd.tile([P, D], F32, name=f"x2_{m}")
                    nc.vector.tensor_tensor(x2[:], aors[:], xslt[:], ADD)

                    # LN2 stats (free-axis reductions, natural layout)
                    s1c = phd.tile([P, 1], F32, name=f"s1c{m}")
                    s2c = phd.tile([P, 1], F32, name=f"s2c{m}")
                    sqscr = phd.tile([P, D], F32, tag="sqscr", bufs=2)
                    nc.vector.reduce_sum(
                        out=s1c[:], in_=x2[:], axis=mybir.AxisListType.X
                    )
                    nc.scalar.activation(
                        sqscr[:], x2[:], AF.Square, accum_out=s2c[:]
                    )
                    mu2 = phd.tile([P, 1], F32, name=f"mu2_{m}")
                    nc.vector.tensor_scalar_mul(mu2[:], s1c[:], 1.0 / D)
                    v2 = phd.tile([P, 1], F32, name=f"v2_{m}")
                    nc.vector.tensor_tensor(v2[:], s1c[:], mu2[:], MULT)
                    nc.vector.tensor_scalar_mul(v2[:], v2[:], -1.0)
                    nc.vector.tensor_tensor(v2[:], v2[:], s2c[:], ADD)
                    nc.vector.tensor_scalar_mul(v2[:], v2[:], 1.0 / (D - 1))
                    std2 = phd.tile([P, 1], F32, name=f"std2_{m}")
                    nc.scalar.activation(std2[:], v2[:], AF.Sqrt)
                    nc.vector.tensor_scalar_add(std2[:], std2[:], EPS)
                    r2 = phd.tile([P, 1], F32, name=f"r2_{m}")
                    nc.vector.reciprocal(r2[:], std2[:])

                    n2 = phd.tile([P, D], F32, name=f"n2_{m}")
                    nc.vector.tensor_tensor(
                        n2[:], x2[:], mu2[:].to_broadcast((P, D)), SUB
                    )
                    nc.vector.tensor_tensor(
                        n2[:], n2[:], r2[:].to_broadcast((P, D)), MULT
                    )
                    aorss.append(aors)
                    x2s.append(x2)
                    n2s.append(n2)

                # transpose n2 -> n2T [D, tok], per token half
                ident = phd.tile([P, P], F32)
                make_identity(nc, ident[:])
                n2Ts = []
                for m in range(TM):
                    n2T = phd.tile([P, KS, P], BF16, name=f"n2T{m}")
                    for i in range(KS):
                        ps_t = psD.tile([P, P], F32, name="ps_t", bufs=2)
                        nc.tensor.transpose(
                            ps_t[:], n2s[m][:, i * P:(i + 1) * P], ident[:]
                        )
                        nc.vector.tensor_copy(n2T[:, i, :], ps_t[:])
                    n2Ts.append(n2T)

                # ff1T = (W1*a2)^T @ n2T, relu; split by token half so the
                # first half runs while the second RS is still in flight
                bias1t = phd.tile([P, FFS], F32)
                nc.sync.dma_start(bias1t[:], bias1[:])
                reluT = phd.tile([P, FFA, TS], BF16)
                nc.sync.dma_start(reluT[:, FFS, :], fftail[:])
                for g in range(8):  # stream W1 in 512-col groups
                    w1t = phd.tile([P, KS, 512], BF16, tag="w1t", bufs=3)
                    nc.sync.dma_start(w1t[:], w1[:, :, 512 * g:512 * g + 512])
                    for i4 in range(4):
                        i = 4 * g + i4
                        for m in range(TM):
                            ps_f = psD.tile([P, P], F32, name="ps_f", bufs=2)
                            for k in range(KS):
                                nc.tensor.matmul(
                                    ps_f[:],
                                    w1t[:, k, 128 * i4:128 * i4 + 128],
                                    n2Ts[m][:, k, :],
                                    start=(k == 0), stop=(k == KS - 1),
                                )
                            nc.scalar.activation(
                                reluT[:, i, m * P:(m + 1) * P], ps_f[:],
                                AF.Relu, bias=bias1t[:, i:i + 1],
                            )

                # ff2 + B2 (aug row) + residual; W2 streamed per k-subtile
                ps_o0 = psD.tile([P, D], F32, name="ps_o0", tag="ps_big",
                                 bufs=2)
                ps_o1 = psD.tile([P, D], F32, name="ps_o1", tag="ps_big",
                                 bufs=2)
                ps_os = [ps_o0, ps_o1]
                for k in range(FFA):
                    w2t = phd.tile([P, D], BF16, tag="w2t", bufs=8)
                    nc.sync.dma_start(w2t[:], w2[:, k, :])
                    for m in range(TM):
                        for nb in range(2):
                            ds_ = slice(512 * nb, 512 * nb + 512)
                            nc.tensor.matmul(
                                ps_os[m][:, ds_],
                                reluT[:, k, m * P:(m + 1) * P],
                                w2t[:, ds_],
                                start=(k == 0), stop=(k == FFA - 1),
                            )
                for m in range(TM):
                    outt = phd.tile([P, D], F32, name=f"outt{m}")
                    nc.vector.tensor_tensor(
                        outt[:], ps_os[m][:], x2s[m][:], ADD
                    )
                    nc.sync.dma_start(
                        out[:].rearrange("(m p) d -> p m d", p=P)[:, m, :],
                        outt[:],
                    )
    nc.compile()
    return nc


def _prep_inputs(x, mask, Wq, Wo, W1, B1, W2, B2, ln1_a, ln1_b, ln2_a, ln2_b):
    """Host-side folding + striping. Returns in_maps for 8 cores."""
    f32 = np.float32

    def strip(a, ks):  # [ks*128, F] -> [128, ks, F]
        return np.ascontiguousarray(
            a.reshape(ks, P, -1).transpose(1, 0, 2).astype(f32)
        )

    Wa = (Wq * ln1_a[:, None]).astype(f32)          # LN1 scale folded
    g = Wa.sum(axis=0)                               # [D]
    c1 = (Wq.T @ ln1_b).astype(f32)                  # [D]
    Wa1 = (W1 * ln2_a[:, None]).astype(f32)
    bias1_full = (B1 + W1.T @ ln2_b).astype(f32)     # [DFF]

    w1_s = strip(Wa1, KS).astype(ml_dtypes.bfloat16)  # [128, 8, 4096]
    w2_aug = np.zeros((FFA * P, D), f32)
    w2_aug[:DFF] = W2
    w2_aug[DFF] = B2
    w2_s = strip(w2_aug, FFA).astype(ml_dtypes.bfloat16)  # [128, 33, 1024]
    bias1_s = np.ascontiguousarray(bias1_full.reshape(FFS, P).T)  # [128, 32]

    ones1 = np.ones((1, P), f32)
    sel4 = np.zeros((HD, HCOLS), f32)
    for h in range(HD):
        sel4[h, 64 * h:64 * h + 64] = 1.0
    fftail = np.zeros((P, TS), ml_dtypes.bfloat16)
    fftail[0] = 1.0

    in_maps = []
    for c in range(NC):
        b, j = divmod(c, 4)
        cols = slice(HCOLS * j, HCOLS * j + HCOLS)
        toks = slice(TS * j, TS * j + TS)

        xt_aug = np.zeros((P, KA, S), ml_dtypes.bfloat16)
        xt_aug[:, :KS] = strip(np.ascontiguousarray(x[b].T), KS)
        wq_aug = np.zeros((P, KA, HCOLS), ml_dtypes.bfloat16)
        wq_aug[:, :KS] = strip(Wa[:, cols], KS)
        wq_aug[0, KS] = g[cols]
        c1_s = np.ascontiguousarray(c1[cols])
        c1c = np.ascontiguousarray(c1_s.reshape(HM, P).T)  # [128, 2]
        maskt = np.ascontiguousarray(mask[b, 0].T).astype(f32)

        in_maps.append({
            "xt": xt_aug,
            "wq": wq_aug,
            "ones1": ones1,
            "sel4": sel4,
            "c1r": c1_s.reshape(1, HCOLS),
            "c1c": c1c,
            "xsl": strip(np.ascontiguousarray(x[b, toks]), TM),
            "maskt": strip(maskt, KS).astype(ml_dtypes.bfloat16),
            "wo": strip(
                np.ascontiguousarray(np.asarray(Wo, f32)[cols]), HM
            ).astype(ml_dtypes.bfloat16),
            "w1": w1_s,
            "w2": w2_s,
            "bias1": bias1_s,
            "fftail": fftail,
        })
    return in_maps


def kernel(**inputs):
    if "nc" not in _CACHE:
        _CACHE["nc"] = _build()
    nc = _CACHE["nc"]
    args = {k: np.asarray(v) for k, v in inputs.items()}
    in_maps = _prep_inputs(
        args["x"], args["mask"], args["Wq"], args["Wo"], args["W1"],
        args["B1"], args["W2"], args["B2"], args["ln1_a"], args["ln1_b"],
        args["ln2_a"], args["ln2_b"],
    )
    res = bass_utils.run_bass_kernel_spmd(
        nc, in_maps, core_ids=list(range(NC))
    )
    out = np.empty((B, S, D), np.float32)
    for c in range(NC):
        b, j = divmod(c, 4)
        out[b, TS * j:TS * j + TS] = res.results[c]["out"]
    return out
